# revision 25
# baseline (speedup 1.0000x reference)
"""NonLocalBlock (B=4, C=64, Ci=32, H=W=64) on 8 TRN2 NeuronCores.

Sharding: data-parallel over batch (4 pairs of cores); within each pair
the query dimension n of the NxN score matrix is split in half.
Softmax runs over n (dim=1): each core computes partial denominators
S[m] over its n-half; partners exchange partials with a direct
remote-DMA pair write (SBUF->SBUF + remote semaphore, tpb^=1), a few us
instead of the ~18us collective_compute latency floor.

Per core (b = core//2, h = core%2):
  phase A (PE warm-up, back-to-back):
    theta = theta_aug^T @ supp_aug   -> fp8 DoubleRow [16,2,512] per n-chunk
    phi   = phi_aug^T @ ref_aug      -> fp8 DR, replicated to 4 quadrants
    wgt   = ref_aug^T @ (w_w@g_w | w_w@g_b)  [128,64] f32 per m-tile (x32)
  phase B per m-tile (x32):
    fT    = phi_dr^T @ theta_dr  (fp8 DoubleRow, 4x [128,512] PSUM f32)
    exp   on ACT (native Exp + accum row-sums) or DVE (Schraudolph:
          int16(f*128/ln2 + 16250.15) bitcast bf16 + tensor_reduce rowsum)
    every 4 tiles: pair-exchange partial sums via remote_dma_broadcast;
          stot = local + recv; 1/stot scales wgt -> bf16 (gpsimd)
    z    += wgt_b16^T @ expT   (PSUM accum [64,2048], interleaved)
  epilogue: out = z + w_b + supp  (DVE/Pool split), DMA out
"""

import numpy as np

B, C, CI, H, W = 4, 64, 32, 64, 64
N = H * W            # 4096
NLOC = N // 2        # 2048 n-columns per core
NCORES = 8
MTP = 128            # m-tile partition size
NMT = N // MTP       # 32 m-tiles
GROUPS = [6, 10, 10, 6]      # m-tiles per softmax-sum exchange group
CK = 512             # matmul moving-dim chunk (one psum bank of f32)
REPLICA_GROUPS = [[0, 1], [2, 3], [4, 5], [6, 7]]

# emission-order timing model (us): the PE executes in program order, so
# CC-dependent work must not be emitted before its AllReduce has (by
# estimate) landed, or the engine FIFOs stall.
A_END = 10.0
TILE_T = 1.52
CC_LAT = 18.0
CC_GAP = 10.0

# Schraudolph exp in bf16: i16 = round(f*128/ln2 + (127*128 - Cmagic))
SCHRAUD_A = 128.0 / float(np.log(2.0))
SCHRAUD_B = 16256.0 - 5.85

_cache = {}


def _build():
    import concourse.bacc as bacc
    import concourse.tile as tile
    from concourse import mybir

    f32 = mybir.dt.float32
    bf16 = mybir.dt.bfloat16
    fp8 = mybir.dt.float8e4
    i16 = mybir.dt.int16
    AF = mybir.ActivationFunctionType
    ALU = mybir.AluOpType
    PM = mybir.MatmulPerfMode

    nc = bacc.Bacc(None, target_bir_lowering=False, debug=False)

    suppa = nc.dram_tensor("suppa", [C + 1, NLOC], bf16, kind="ExternalInput")
    supp32 = nc.dram_tensor("supp32", [C, NLOC], f32, kind="ExternalInput")
    ref_aug = nc.dram_tensor("ref_aug", [C + 1, N], bf16, kind="ExternalInput")
    theta_aug = nc.dram_tensor("theta_aug", [C + 1, CI], bf16, kind="ExternalInput")
    phi_aug = nc.dram_tensor("phi_aug", [C + 1, CI], bf16, kind="ExternalInput")
    wg_aug = nc.dram_tensor("wg_aug", [C + 1, C], bf16, kind="ExternalInput")
    w_bc = nc.dram_tensor("w_bc", [C, 1], f32, kind="ExternalInput")
    out = nc.dram_tensor("out", [C, NLOC], f32, kind="ExternalOutput")

    with tile.TileContext(nc) as tc:
        from contextlib import ExitStack

        with ExitStack() as ctx, nc.allow_low_precision("fp8/bf16 softmax approx"):
            sing = ctx.enter_context(tc.tile_pool(name="sing", bufs=1))
            epool = ctx.enter_context(tc.tile_pool(name="expT", bufs=NMT))
            outp = ctx.enter_context(tc.tile_pool(name="outp", bufs=3))
            dpool = ctx.enter_context(
                tc.tile_pool(name="dram", bufs=len(GROUPS), space="DRAM")
            )

            # ---------------- loads ----------------
            suppa_t = sing.tile([C + 1, NLOC], bf16, tag="suppa")
            nc.sync.dma_start(out=suppa_t[0:33, :], in_=suppa[0:33, :])
            nc.scalar.dma_start(out=suppa_t[33:, :], in_=suppa[33:, :])
            taw = sing.tile([C + 1, CI], bf16, tag="taw")
            nc.gpsimd.dma_start(out=taw, in_=theta_aug[:, :])
            paw = sing.tile([C + 1, CI], bf16, tag="paw")
            nc.gpsimd.dma_start(out=paw, in_=phi_aug[:, :])
            wga = sing.tile([C + 1, C], bf16, tag="wga")
            nc.gpsimd.dma_start(out=wga, in_=wg_aug[:, :])
            wb = sing.tile([C, 1], f32, tag="wb")
            nc.gpsimd.dma_start(out=wb, in_=w_bc[:, :])
            refa = sing.tile([C + 1, N], bf16, tag="refa")
            nc.sync.dma_start(out=refa[0:33, :], in_=ref_aug[0:33, :])
            nc.scalar.dma_start(out=refa[33:, :], in_=ref_aug[33:, :])
            supp_t = sing.tile([C, NLOC], f32, tag="supp32")
            nc.gpsimd.dma_start(out=supp_t[0:32, :], in_=supp32[0:32, :])
            nc.gpsimd.dma_start(out=supp_t[32:, :], in_=supp32[32:, :])

            # fp8 DoubleRow operands (Ci pairs (i, i+16) -> partition i,
            # block b), all anchored at partitions 0-15.
            fp8_raw = sing.tile([MTP, 4, 2 * CK], fp8, tag="fp8_raw")
            theta_dr = sing.tile([16, 2, 4 * CK], fp8, tag="theta_dr")
            phi_dr = sing.tile([16, 2, 8 * CK], fp8, tag="phi_dr")

            wgt_b16u = sing.tile([MTP, NMT * C], bf16, tag="wgtb16u")
            wgt_b16 = sing.tile([MTP, NMT * C], bf16, tag="wgtb16")
            # unit row-sums: col = h*NMT + mt
            sAB = sing.tile([MTP, 2 * NMT], f32, tag="sAB")
            sloc = sing.tile([MTP, NMT], f32, tag="sloc")
            ssum = sing.tile([MTP, NMT], f32, tag="ssum")
            srec = sing.tile([MTP, NMT], f32, tag="srec")

            # ---------------- phase A: projections + wgt ----------------
            projA_ctx = ExitStack()
            projA = projA_ctx.enter_context(
                tc.tile_pool(name="projA", bufs=2, space="PSUM")
            )

            # 12 projection chunks (theta 0-3 from supp, phi 4-11 from ref),
            # 3 per PSUM tile at quadrants {0, 32, 64}, 2 blk-matmuls each.
            engs = [nc.sync, nc.scalar, nc.gpsimd]
            pt = None
            for c in range(12):
                q = c % 3
                t = c // 3
                if q == 0:
                    pt = projA.tile(
                        [MTP, 2, CK], f32, tag="projps", name=f"proj_ps{t}"
                    )
                if c < 4:
                    lw, rt = taw, suppa_t[:, c * CK : (c + 1) * CK]
                else:
                    m = c - 4
                    lw, rt = paw, refa[:, m * CK : (m + 1) * CK]
                for blk in range(2):
                    nc.tensor.matmul(
                        pt[32 * q : 32 * q + 16, blk : blk + 1, :],
                        lhsT=lw[:, 16 * blk : 16 * blk + 16],
                        rhs=rt,
                        start=True,
                        stop=True,
                    )
                if q == 2:
                    if t % 2 == 0:
                        nc.vector.tensor_copy(fp8_raw[:, t : t + 1, :], pt)
                    else:
                        nc.scalar.copy(fp8_raw[:, t : t + 1, :], pt)
            # assemble DR operands via SBUF->SBUF DMAs (partition remap)
            for c in range(12):
                src = fp8_raw[32 * (c % 3) : 32 * (c % 3) + 16, c // 3 : c // 3 + 1, :]
                if c < 4:
                    dst = theta_dr[0:16, :, c * CK : (c + 1) * CK]
                else:
                    m = c - 4
                    dst = phi_dr[0:16, :, m * CK : (m + 1) * CK]
                engs[c % 3].dma_start(out=dst, in_=src)

            # wgt matmuls keep the PE stream dense (HAM stays warm)
            projA_ctx.close()
            ftp = ctx.enter_context(tc.tile_pool(name="ftp", bufs=2, space="PSUM"))
            wgtA_ctx = ExitStack()
            wgtA = wgtA_ctx.enter_context(
                tc.tile_pool(name="wgtA", bufs=2, space="PSUM")
            )
            for g4 in range(NMT // 4):
                wps = wgtA.tile([MTP, 4 * C], f32, tag="wgtps")
                for k in range(4):
                    mt = 4 * g4 + k
                    nc.tensor.matmul(
                        wps[:, k * C : (k + 1) * C],
                        lhsT=refa[:, mt * MTP : (mt + 1) * MTP],
                        rhs=wga[:, :],
                        start=True,
                        stop=True,
                    )
                nc.vector.tensor_copy(
                    wgt_b16u[:, g4 * 4 * C : (g4 + 1) * 4 * C], wps
                )
            wgtA_ctx.close()
            zpp = ctx.enter_context(tc.tile_pool(name="zpp", bufs=1, space="PSUM"))
            z_ps = zpp.tile([C, NLOC], f32, tag="z", name="z_ps")

            # ---------------- phase B ----------------
            ets = [None] * NMT
            est = {"A": A_END + 1.3, "D": A_END + 0.7}  # ACT tbl; DVE drain
            UC = {"A": 1.20, "D": 1.57}  # per-unit cost estimate (us)
            gstart = [sum(GROUPS[:g]) for g in range(len(GROUPS))]
            gend = [gstart[g] + GROUPS[g] for g in range(len(GROUPS))]
            group_of = []
            for g, gn in enumerate(GROUPS):
                group_of += [g] * gn
            cc_land = [None] * len(GROUPS)
            z_tiles_ready = []
            pending_land = []

            def emit_z(budget):
                while z_tiles_ready and budget > 0:
                    t = z_tiles_ready.pop(0)
                    for j in range(NLOC // CK):
                        nc.tensor.matmul(
                            z_ps[:, j * CK : (j + 1) * CK],
                            lhsT=wgt_b16[:, t * C : (t + 1) * C],
                            rhs=ets[t][:, j * CK : (j + 1) * CK],
                            start=(t == 0),
                            stop=(t == NMT - 1),
                        )
                    budget -= 1

            def land_group(g):
                # emit the CC-dependent tail for group g: reciprocal + wgt
                # scales. Only call once cc_land[g] <= estimated clock, or
                # the DVE FIFO stalls on the unlanded AllReduce.
                gsl = slice(gstart[g], gend[g])
                nc.vector.reciprocal(out=srec[:, gsl], in_=ssum[:, gsl])
                est["D"] += 0.15
                for t in range(gstart[g], gend[g]):
                    nc.vector.tensor_scalar_mul(
                        wgt_b16[:, t * C : (t + 1) * C],
                        wgt_b16u[:, t * C : (t + 1) * C],
                        srec[:, t : t + 1],
                    )
                    est["D"] += 0.1
                z_tiles_ready.extend(range(gstart[g], gend[g]))

            for mt in range(NMT):
                et = epool.tile([MTP, NLOC], bf16, tag="et")
                ets[mt] = et
                for hh in range(2):
                    ft = ftp.tile([MTP, 2 * CK], f32, tag="ft")
                    for jj in range(2):
                        jn = 2 * hh + jj
                        nc.tensor.matmul(
                            ft[:, jj * CK : (jj + 1) * CK],
                            lhsT=phi_dr[0:16, :, mt * MTP : (mt + 1) * MTP],
                            rhs=theta_dr[0:16, :, jn * CK : (jn + 1) * CK],
                            start=True,
                            stop=True,
                            perf_mode=PM.DoubleRow,
                        )
                    scol = hh * NMT + mt
                    eng = "A" if est["A"] + UC["A"] <= est["D"] + UC["D"] else "D"
                    est[eng] += UC[eng]
                    if eng == "A":
                        nc.scalar.activation(
                            out=et[:, hh * 2 * CK : (hh + 1) * 2 * CK],
                            in_=ft,
                            func=AF.Exp,
                            accum_out=sAB[:, scol : scol + 1],
                        )
                    else:
                        nc.vector.tensor_scalar(
                            out=et[:, hh * 2 * CK : (hh + 1) * 2 * CK].bitcast(i16),
                            in0=ft,
                            scalar1=SCHRAUD_A,
                            scalar2=SCHRAUD_B,
                            op0=ALU.mult,
                            op1=ALU.add,
                        )
                        nc.vector.tensor_reduce(
                            out=sAB[:, scol : scol + 1],
                            in_=et[:, hh * 2 * CK : (hh + 1) * 2 * CK],
                            axis=mybir.AxisListType.X,
                            op=ALU.add,
                        )

                g = group_of[mt]
                if mt == gend[g] - 1:
                    gsl = slice(gstart[g], gend[g])
                    gn = GROUPS[g]
                    # local sum of both halves (gpsimd: SBUF->SBUF is fine)
                    nc.gpsimd.tensor_add(
                        sloc[:, gsl],
                        sAB[:, gstart[g] : gend[g]],
                        sAB[:, NMT + gstart[g] : NMT + gend[g]],
                    )
                    cin = dpool.tile([MTP, gn], f32, tag=f"cin{g}")
                    cout = dpool.tile([MTP, gn], f32, tag=f"cout{g}")
                    nc.gpsimd.dma_start(out=cin, in_=sloc[:, gsl])
                    nc.gpsimd.collective_compute(
                        "AllReduce",
                        ALU.add,
                        replica_groups=REPLICA_GROUPS,
                        ins=[cin.opt()],
                        outs=[cout.opt()],
                    )
                    # sync queue stalls on the CC sem; it is otherwise idle
                    nc.sync.dma_start(out=ssum[:, gsl], in_=cout)
                    issue = max(est["A"], est["D"])
                    cc_land[g] = max(
                        issue + CC_LAT,
                        (cc_land[g - 1] + CC_GAP) if g else 0.0,
                    )
                    pending_land.append(g)

                clk = max(est["A"], est["D"])
                while pending_land and cc_land[pending_land[0]] <= clk:
                    land_group(pending_land.pop(0))
                emit_z(5)

            while pending_land:
                land_group(pending_land.pop(0))
            emit_z(len(z_tiles_ready))

            # ---------------- epilogue ----------------
            for j in range(NLOC // CK):
                e2 = outp.tile([C, CK], f32, tag="e2")
                eng = nc.vector
                eng.scalar_tensor_tensor(
                    out=e2,
                    in0=z_ps[:, j * CK : (j + 1) * CK],
                    scalar=wb[:, :],
                    in1=supp_t[:, j * CK : (j + 1) * CK],
                    op0=ALU.add,
                    op1=ALU.add,
                )
                deng = nc.sync if j % 2 == 0 else nc.scalar
                deng.dma_start(out=out[:, j * CK : (j + 1) * CK], in_=e2)

    nc.compile()
    return nc


def _get_nc():
    if "nc" not in _cache:
        _cache["nc"] = _build()
    return _cache["nc"]


def kernel(
    supp_feature,
    ref_feature,
    theta_w,
    theta_b,
    phi_w,
    phi_b,
    g_w,
    g_b,
    w_w,
    w_b,
    _trace=False,
):
    import ml_dtypes

    try:
        import antenv.axon_hooks  # noqa: F401
    except ImportError:
        import sys
        import types

        import antenv

        _mod = types.ModuleType("antenv.axon_hooks")
        _mod._hook = None
        _mod.get_axon_ntff_profile_hook = lambda: _mod._hook
        _mod.set_axon_ntff_profile_hook = lambda h: setattr(_mod, "_hook", h)
        sys.modules["antenv.axon_hooks"] = _mod
        antenv.axon_hooks = _mod

    from concourse.bass_utils import run_bass_kernel_spmd

    bf = ml_dtypes.bfloat16
    supp_feature = np.asarray(supp_feature, dtype=np.float32)
    ref_feature = np.asarray(ref_feature, dtype=np.float32)
    theta_w = np.asarray(theta_w, dtype=np.float32)
    theta_b = np.asarray(theta_b, dtype=np.float32)
    phi_w = np.asarray(phi_w, dtype=np.float32)
    phi_b = np.asarray(phi_b, dtype=np.float32)
    g_w = np.asarray(g_w, dtype=np.float32)
    g_b = np.asarray(g_b, dtype=np.float32)
    w_w = np.asarray(w_w, dtype=np.float32)
    w_b = np.asarray(w_b, dtype=np.float32)

    nc = _get_nc()

    supp2 = supp_feature.reshape(B, C, N)
    ref2 = ref_feature.reshape(B, C, N)
    # Fold the output 1x1 conv into g: w_w@(g_w@ref+g_b) = (w_w@g_w)@ref + w_w@g_b
    Wg = (w_w @ g_w).astype(np.float32)
    wgb = (w_w @ g_b).astype(np.float32)
    wg_aug = np.ascontiguousarray(
        np.concatenate([Wg.T, wgb[None, :]], axis=0).astype(bf)
    )
    theta_augh = np.ascontiguousarray(
        np.concatenate([theta_w.T, theta_b[None, :]], axis=0).astype(bf)
    )
    phi_augh = np.ascontiguousarray(
        np.concatenate([phi_w.T, phi_b[None, :]], axis=0).astype(bf)
    )

    in_maps = []
    for core in range(NCORES):
        b, h = core // 2, core % 2
        ref_augh = np.ascontiguousarray(
            np.concatenate([ref2[b], np.ones((1, N), np.float32)], axis=0).astype(bf)
        )
        sl = supp2[b, :, h * NLOC : (h + 1) * NLOC]
        suppa_h = np.ascontiguousarray(
            np.concatenate([sl, np.ones((1, NLOC), np.float32)], axis=0).astype(bf)
        )
        in_maps.append(
            {
                "suppa": suppa_h,
                "supp32": np.ascontiguousarray(sl),
                "ref_aug": ref_augh,
                "theta_aug": theta_augh,
                "phi_aug": phi_augh,
                "wg_aug": wg_aug,
                "w_bc": np.ascontiguousarray(w_b.reshape(C, 1)),
            }
        )

    res = run_bass_kernel_spmd(nc, in_maps, list(range(NCORES)), trace=_trace)
    if _trace:
        _cache["last_exec_time_ns"] = res.exec_time_ns
        _cache["last_results"] = res

    z = np.empty((B, C, N), dtype=np.float32)
    for core in range(NCORES):
        b, h = core // 2, core % 2
        z[b, :, h * NLOC : (h + 1) * NLOC] = res.results[core]["out"]
    return z.reshape(B, C, H, W)


# revision 32
# speedup vs baseline: 1.1559x; 1.1559x over previous
"""NonLocalBlock (B=4, C=64, Ci=32, H=W=64) on 8 TRN2 NeuronCores.

Sharding: data-parallel over batch (4 pairs of cores); within each pair
the query dimension n of the NxN score matrix is split in half.
Softmax runs over n (dim=1): each core computes partial denominators
S[m] over its n-half; partners exchange partials with a direct
remote-DMA pair write (SBUF->SBUF + remote semaphore, tpb^=1), a few us
instead of the ~18us collective_compute latency floor.

Per core (b = core//2, h = core%2):
  phase A (PE warm-up, back-to-back):
    theta = theta_aug^T @ supp_aug   -> fp8 DoubleRow [16,2,512] per n-chunk
    phi   = phi_aug^T @ ref_aug      -> fp8 DR, replicated to 4 quadrants
    wgt   = ref_aug^T @ (w_w@g_w | w_w@g_b)  [128,64] f32 per m-tile (x32)
  phase B per m-tile (x32):
    fT    = phi_dr^T @ theta_dr  (fp8 DoubleRow, 4x [128,512] PSUM f32)
    exp   on ACT (native Exp + accum row-sums) or DVE (Schraudolph:
          int16(f*128/ln2 + 16250.15) bitcast bf16 + tensor_reduce rowsum)
    every 4 tiles: pair-exchange partial sums via remote_dma_broadcast;
          stot = local + recv; 1/stot scales wgt -> bf16 (gpsimd)
    z    += wgt_b16^T @ expT   (PSUM accum [64,2048], interleaved)
  epilogue: out = z + w_b + supp  (DVE/Pool split), DMA out
"""

import numpy as np

B, C, CI, H, W = 4, 64, 32, 64, 64
N = H * W            # 4096
NLOC = N // 2        # 2048 n-columns per core
NCORES = 8
MTP = 128            # m-tile partition size
NMT = N // MTP       # 32 m-tiles
GROUPS = [8, 10, 10, 4]      # m-tiles per softmax-sum exchange group
CK = 512             # matmul moving-dim chunk (one psum bank of f32)
REPLICA_GROUPS = [[0, 1], [2, 3], [4, 5], [6, 7]]

# emission-order timing model (us): the PE executes in program order, so
# CC-dependent work must not be emitted before its AllReduce has (by
# estimate) landed, or the engine FIFOs stall.
A_END = 12.0
CC_LAT = 18.0
CC_GAP = 10.0

# Schraudolph exp in bf16: i16 = round(f*128/ln2 + (127*128 - Cmagic))
SCHRAUD_A = 128.0 / float(np.log(2.0))
SCHRAUD_B = 16256.0 - 5.85

_cache = {}


def _build():
    import concourse.bacc as bacc
    import concourse.tile as tile
    from concourse import mybir

    f32 = mybir.dt.float32
    bf16 = mybir.dt.bfloat16
    fp8 = mybir.dt.float8e4
    i16 = mybir.dt.int16
    AF = mybir.ActivationFunctionType
    ALU = mybir.AluOpType
    PM = mybir.MatmulPerfMode

    nc = bacc.Bacc(None, target_bir_lowering=False, debug=False)

    suppa = nc.dram_tensor("suppa", [C + 1, NLOC], bf16, kind="ExternalInput")
    supp32 = nc.dram_tensor("supp32", [C, NLOC], f32, kind="ExternalInput")
    ref_aug = nc.dram_tensor("ref_aug", [C + 1, N], bf16, kind="ExternalInput")
    theta_aug = nc.dram_tensor("theta_aug", [C + 1, CI], bf16, kind="ExternalInput")
    phi_aug = nc.dram_tensor("phi_aug", [C + 1, CI], bf16, kind="ExternalInput")
    wg_aug = nc.dram_tensor("wg_aug", [C + 1, C], bf16, kind="ExternalInput")
    w_bc = nc.dram_tensor("w_bc", [C, 1], f32, kind="ExternalInput")
    out = nc.dram_tensor("out", [C, NLOC], f32, kind="ExternalOutput")

    with tile.TileContext(nc) as tc:
        from contextlib import ExitStack

        with ExitStack() as ctx, nc.allow_low_precision("fp8/bf16 softmax approx"):
            sing = ctx.enter_context(tc.tile_pool(name="sing", bufs=1))
            epool = ctx.enter_context(tc.tile_pool(name="expT", bufs=NMT))
            outp = ctx.enter_context(tc.tile_pool(name="outp", bufs=3))
            dpool = ctx.enter_context(
                tc.tile_pool(name="dram", bufs=len(GROUPS), space="DRAM")
            )

            # ---------------- loads ----------------
            suppa_t = sing.tile([C + 1, NLOC], bf16, tag="suppa")
            nc.sync.dma_start(out=suppa_t[0:33, :], in_=suppa[0:33, :])
            nc.scalar.dma_start(out=suppa_t[33:, :], in_=suppa[33:, :])
            taw = sing.tile([C + 1, CI], bf16, tag="taw")
            nc.gpsimd.dma_start(out=taw, in_=theta_aug[:, :])
            paw = sing.tile([C + 1, CI], bf16, tag="paw")
            nc.gpsimd.dma_start(out=paw, in_=phi_aug[:, :])
            wga = sing.tile([C + 1, C], bf16, tag="wga")
            nc.gpsimd.dma_start(out=wga, in_=wg_aug[:, :])
            wb = sing.tile([C, 1], f32, tag="wb")
            nc.gpsimd.dma_start(out=wb, in_=w_bc[:, :])
            refa = sing.tile([C + 1, N], bf16, tag="refa")
            nc.sync.dma_start(out=refa[0:33, :], in_=ref_aug[0:33, :])
            nc.scalar.dma_start(out=refa[33:, :], in_=ref_aug[33:, :])
            supp_t = sing.tile([C, NLOC], f32, tag="supp32")
            nc.gpsimd.dma_start(out=supp_t[0:32, :], in_=supp32[0:32, :])
            nc.gpsimd.dma_start(out=supp_t[32:, :], in_=supp32[32:, :])

            # bf16 projection outputs, anchored at partitions 0-31
            prj_raw = sing.tile([96, 4, CK], bf16, tag="prj_raw")
            theta_b = sing.tile([CI, 4 * CK], bf16, tag="theta_b")
            phi_b = sing.tile([CI, 8 * CK], bf16, tag="phi_b")

            wgt_b16u = sing.tile([MTP, NMT * C], bf16, tag="wgtb16u")
            wgt_b16 = sing.tile([MTP, NMT * C], bf16, tag="wgtb16")
            # unit row-sums: col = h*NMT + mt
            sAB = sing.tile([MTP, 2 * NMT], f32, tag="sAB")
            sloc = sing.tile([MTP, NMT], f32, tag="sloc")
            ssum = sing.tile([MTP, NMT], f32, tag="ssum")
            srec = sing.tile([MTP, NMT], f32, tag="srec")

            # ---------------- phase A: projections + wgt ----------------
            projA_ctx = ExitStack()
            projA = projA_ctx.enter_context(
                tc.tile_pool(name="projA", bufs=3, space="PSUM")
            )

            # 12 projection chunks (theta 0-3 from supp, phi 4-11 from ref),
            # 3 per PSUM tile at quadrants {0, 32, 64} for cheap batched
            # copies; assembly DMAs remap to partitions 0-31.
            engs = [nc.sync, nc.scalar, nc.gpsimd]
            pt = None
            for c in range(12):
                q = c % 3
                t = c // 3
                if q == 0:
                    pt = projA.tile([96, CK], f32, tag="projps", name=f"proj_ps{t}")
                if c < 4:
                    lw, rt = taw, suppa_t[:, c * CK : (c + 1) * CK]
                else:
                    m = c - 4
                    lw, rt = paw, refa[:, m * CK : (m + 1) * CK]
                nc.tensor.matmul(
                    pt[32 * q : 32 * q + 32, :],
                    lhsT=lw[:, :],
                    rhs=rt,
                    start=True,
                    stop=True,
                )
                if q == 2:
                    if t % 2 == 0:
                        nc.vector.tensor_copy(prj_raw[:, t : t + 1, :], pt)
                    else:
                        nc.scalar.copy(prj_raw[:, t : t + 1, :], pt)
            # assemble contiguous [32, n] operands via SBUF->SBUF DMAs
            for c in range(12):
                src = prj_raw[32 * (c % 3) : 32 * (c % 3) + 32, c // 3 : c // 3 + 1, :]
                if c < 4:
                    dst = theta_b[:, c * CK : (c + 1) * CK]
                else:
                    m = c - 4
                    dst = phi_b[:, m * CK : (m + 1) * CK]
                engs[c % 3].dma_start(out=dst, in_=src)

            # wgt matmuls keep the PE stream dense (HAM stays warm)
            projA_ctx.close()
            ftp = ctx.enter_context(tc.tile_pool(name="ftp", bufs=2, space="PSUM"))
            wgtA_ctx = ExitStack()
            wgtA = wgtA_ctx.enter_context(
                tc.tile_pool(name="wgtA", bufs=2, space="PSUM")
            )
            for g4 in range(NMT // 4):
                wps = wgtA.tile([MTP, 4 * C], f32, tag="wgtps")
                for k in range(4):
                    mt = 4 * g4 + k
                    nc.tensor.matmul(
                        wps[:, k * C : (k + 1) * C],
                        lhsT=refa[:, mt * MTP : (mt + 1) * MTP],
                        rhs=wga[:, :],
                        start=True,
                        stop=True,
                    )
                nc.vector.tensor_copy(
                    wgt_b16u[:, g4 * 4 * C : (g4 + 1) * 4 * C], wps
                )
            wgtA_ctx.close()
            zpp = ctx.enter_context(tc.tile_pool(name="zpp", bufs=1, space="PSUM"))
            z_ps = zpp.tile([C, NLOC], f32, tag="z", name="z_ps")

            # ---------------- phase B ----------------
            ets = [None] * NMT
            est = {"A": A_END + 1.3, "D": A_END + 0.5}  # ACT tbl; DVE drain
            UC = {"A": 1.29, "D": 2.39}  # per-unit cost (DVE incl reduce)
            gstart = [sum(GROUPS[:g]) for g in range(len(GROUPS))]
            gend = [gstart[g] + GROUPS[g] for g in range(len(GROUPS))]
            group_of = []
            for g, gn in enumerate(GROUPS):
                group_of += [g] * gn
            cc_land = [None] * len(GROUPS)
            z_tiles_ready = []
            pending_land = []

            def emit_z(budget):
                while z_tiles_ready and budget > 0:
                    t = z_tiles_ready.pop(0)
                    for j in range(NLOC // CK):
                        nc.tensor.matmul(
                            z_ps[:, j * CK : (j + 1) * CK],
                            lhsT=wgt_b16[:, t * C : (t + 1) * C],
                            rhs=ets[t][:, j * CK : (j + 1) * CK],
                            start=(t == 0),
                            stop=(t == NMT - 1),
                        )
                    budget -= 1

            def land_group(g):
                # emit the CC-dependent tail for group g: reciprocal + wgt
                # scales. Only call once cc_land[g] <= estimated clock, or
                # the DVE FIFO stalls on the unlanded AllReduce.
                gsl = slice(gstart[g], gend[g])
                nc.vector.reciprocal(out=srec[:, gsl], in_=ssum[:, gsl])
                est["D"] += 0.15
                for t in range(gstart[g], gend[g]):
                    nc.vector.tensor_scalar_mul(
                        wgt_b16[:, t * C : (t + 1) * C],
                        wgt_b16u[:, t * C : (t + 1) * C],
                        srec[:, t : t + 1],
                    )
                    est["D"] += 0.1
                z_tiles_ready.extend(range(gstart[g], gend[g]))

            for mt in range(NMT):
                et = epool.tile([MTP, NLOC], bf16, tag="et")
                ets[mt] = et
                for hh in range(2):
                    ft = ftp.tile([MTP, 2 * CK], f32, tag="ft")
                    for jj in range(2):
                        jn = 2 * hh + jj
                        nc.tensor.matmul(
                            ft[:, jj * CK : (jj + 1) * CK],
                            lhsT=phi_b[:, mt * MTP : (mt + 1) * MTP],
                            rhs=theta_b[:, jn * CK : (jn + 1) * CK],
                            start=True,
                            stop=True,
                        )
                    scol = hh * NMT + mt
                    eng = "A" if est["A"] + UC["A"] <= est["D"] + UC["D"] else "D"
                    est[eng] += UC[eng]
                    if eng == "A":
                        nc.scalar.activation(
                            out=et[:, hh * 2 * CK : (hh + 1) * 2 * CK],
                            in_=ft,
                            func=AF.Exp,
                            accum_out=sAB[:, scol : scol + 1],
                        )
                    else:
                        nc.vector.tensor_scalar(
                            out=et[:, hh * 2 * CK : (hh + 1) * 2 * CK].bitcast(i16),
                            in0=ft,
                            scalar1=SCHRAUD_A,
                            scalar2=SCHRAUD_B,
                            op0=ALU.mult,
                            op1=ALU.add,
                        )
                        nc.vector.tensor_reduce(
                            out=sAB[:, scol : scol + 1],
                            in_=et[:, hh * 2 * CK : (hh + 1) * 2 * CK],
                            axis=mybir.AxisListType.X,
                            op=ALU.add,
                        )

                g = group_of[mt]
                if mt == gend[g] - 1:
                    gsl = slice(gstart[g], gend[g])
                    gn = GROUPS[g]
                    # local sum of both halves (gpsimd: SBUF->SBUF is fine)
                    nc.gpsimd.tensor_add(
                        sloc[:, gsl],
                        sAB[:, gstart[g] : gend[g]],
                        sAB[:, NMT + gstart[g] : NMT + gend[g]],
                    )
                    cin = dpool.tile([MTP, gn], f32, tag=f"cin{g}")
                    cout = dpool.tile([MTP, gn], f32, tag=f"cout{g}")
                    nc.gpsimd.dma_start(out=cin, in_=sloc[:, gsl])
                    nc.gpsimd.collective_compute(
                        "AllReduce",
                        ALU.add,
                        replica_groups=REPLICA_GROUPS,
                        ins=[cin.opt()],
                        outs=[cout.opt()],
                    )
                    # sync queue stalls on the CC sem; it is otherwise idle
                    nc.sync.dma_start(out=ssum[:, gsl], in_=cout)
                    issue = max(est["A"], est["D"])
                    cc_land[g] = max(
                        issue + CC_LAT,
                        (cc_land[g - 1] + CC_GAP) if g else 0.0,
                    )
                    pending_land.append(g)

                clk = max(est["A"], est["D"])
                while pending_land and cc_land[pending_land[0]] <= clk:
                    land_group(pending_land.pop(0))
                emit_z(5)

            while pending_land:
                land_group(pending_land.pop(0))
            emit_z(len(z_tiles_ready))

            # ---------------- epilogue ----------------
            for j in range(NLOC // CK):
                e2 = outp.tile([C, CK], f32, tag="e2")
                eng = nc.vector
                eng.scalar_tensor_tensor(
                    out=e2,
                    in0=z_ps[:, j * CK : (j + 1) * CK],
                    scalar=wb[:, :],
                    in1=supp_t[:, j * CK : (j + 1) * CK],
                    op0=ALU.add,
                    op1=ALU.add,
                )
                deng = nc.sync if j % 2 == 0 else nc.scalar
                deng.dma_start(out=out[:, j * CK : (j + 1) * CK], in_=e2)

    nc.compile()
    return nc


def _get_nc():
    if "nc" not in _cache:
        _cache["nc"] = _build()
    return _cache["nc"]


def kernel(
    supp_feature,
    ref_feature,
    theta_w,
    theta_b,
    phi_w,
    phi_b,
    g_w,
    g_b,
    w_w,
    w_b,
    _trace=False,
):
    import ml_dtypes

    try:
        import antenv.axon_hooks  # noqa: F401
    except ImportError:
        import sys
        import types

        import antenv

        _mod = types.ModuleType("antenv.axon_hooks")
        _mod._hook = None
        _mod.get_axon_ntff_profile_hook = lambda: _mod._hook
        _mod.set_axon_ntff_profile_hook = lambda h: setattr(_mod, "_hook", h)
        sys.modules["antenv.axon_hooks"] = _mod
        antenv.axon_hooks = _mod

    from concourse.bass_utils import run_bass_kernel_spmd

    bf = ml_dtypes.bfloat16
    supp_feature = np.asarray(supp_feature, dtype=np.float32)
    ref_feature = np.asarray(ref_feature, dtype=np.float32)
    theta_w = np.asarray(theta_w, dtype=np.float32)
    theta_b = np.asarray(theta_b, dtype=np.float32)
    phi_w = np.asarray(phi_w, dtype=np.float32)
    phi_b = np.asarray(phi_b, dtype=np.float32)
    g_w = np.asarray(g_w, dtype=np.float32)
    g_b = np.asarray(g_b, dtype=np.float32)
    w_w = np.asarray(w_w, dtype=np.float32)
    w_b = np.asarray(w_b, dtype=np.float32)

    nc = _get_nc()

    supp2 = supp_feature.reshape(B, C, N)
    ref2 = ref_feature.reshape(B, C, N)
    # Fold the output 1x1 conv into g: w_w@(g_w@ref+g_b) = (w_w@g_w)@ref + w_w@g_b
    Wg = (w_w @ g_w).astype(np.float32)
    wgb = (w_w @ g_b).astype(np.float32)
    wg_aug = np.ascontiguousarray(
        np.concatenate([Wg.T, wgb[None, :]], axis=0).astype(bf)
    )
    theta_augh = np.ascontiguousarray(
        np.concatenate([theta_w.T, theta_b[None, :]], axis=0).astype(bf)
    )
    phi_augh = np.ascontiguousarray(
        np.concatenate([phi_w.T, phi_b[None, :]], axis=0).astype(bf)
    )

    in_maps = []
    for core in range(NCORES):
        b, h = core // 2, core % 2
        ref_augh = np.ascontiguousarray(
            np.concatenate([ref2[b], np.ones((1, N), np.float32)], axis=0).astype(bf)
        )
        sl = supp2[b, :, h * NLOC : (h + 1) * NLOC]
        suppa_h = np.ascontiguousarray(
            np.concatenate([sl, np.ones((1, NLOC), np.float32)], axis=0).astype(bf)
        )
        in_maps.append(
            {
                "suppa": suppa_h,
                "supp32": np.ascontiguousarray(sl),
                "ref_aug": ref_augh,
                "theta_aug": theta_augh,
                "phi_aug": phi_augh,
                "wg_aug": wg_aug,
                "w_bc": np.ascontiguousarray(w_b.reshape(C, 1)),
            }
        )

    res = run_bass_kernel_spmd(nc, in_maps, list(range(NCORES)), trace=_trace)
    if _trace:
        _cache["last_exec_time_ns"] = res.exec_time_ns
        _cache["last_results"] = res

    z = np.empty((B, C, N), dtype=np.float32)
    for core in range(NCORES):
        b, h = core // 2, core % 2
        z[b, :, h * NLOC : (h + 1) * NLOC] = res.results[core]["out"]
    return z.reshape(B, C, H, W)
